# revision 36
# baseline (speedup 1.0000x reference)
"""Trainium2 Bass kernel for a 2-layer GCN (FCGraphGNN) over 8 NeuronCores.

Math (matches reference):
  ew' = [edge_attr; ones(N)]  (self loops), deg = segsum(ew', dst), dinv = deg^-1/2
  h1 = relu(dinv_dst * segsum(ew * (dinv*x)[src]) @ W1 + b1)
  t2[v] = dinv[v] * h1[v]   (src-side dinv folded into the table)
  h2 = relu(dinv_dst * segsum(ew * t2[src]) @ W2 + b2)
  out = mean-pool-by-graph(h2) @ Wo + bo

Strategy:
  - Shard edges by dst across 8 cores (sorted by dst host-side).
  - Virtual id layout is SLOT-MAJOR: vid = core*NVC + slot*NW + window, so
    every bulk DRAM transfer (ewdeg load, table build/write, h1 staging
    write) is contiguous per partition (no per-row scatter descriptors).
  - NO dinv collective: the degree pass is replicated on every core from
    the full ewdeg table (22MB contiguous load beats a collective here).
  - Layer 1 aggregates raw x (5 features padded to 64 = one 256B gather
    row): no x@W1 preamble. W1 (with b1 folded in as a bias row via a
    constant-1 staging column) applies after aggregation.
  - Layer 1 table T1 = dinv*x built on device: contiguous load, reduce,
    per-chunk tensor_scalar by dinv, contiguous store.
  - Messages gathered with dma_gather (256B rows, 4 SWDGE queues); per
    128-edge tile DVE builds S = (iota==dst_slot)*ew; PE accumulates the
    segment sum in PSUM (bf16 operands).
  - h1 stored bf16, TWO nodes packed per 256B row (tiles segregated by
    src-vid parity; the matmul reads the matching column half). Halves
    the h1 AllGather bytes and the layer-2 gather table.
  - h1 windows accumulate in an SBUF staging tile, ONE contiguous DMA to
    DRAM, AllGather (the only per-iteration collective), then layer 2
    gathers from the global table. Pooled partials AllReduce once.
"""

import os
import sys

import numpy as np

sys.path.insert(0, "/opt/trn_rl_repo")

# ---------------------------------------------------------------- constants
N_NODES = 50000
N_EDGES = 3200000
N_GRAPHS = 50
IN_F = 5
HID = 64
OUT_F = 2
N_CORES = 8

SLOTS = 64          # dst nodes per window
T_SIDE = 17         # 128-edge tiles per (window, src-half)
CAP = T_SIDE * 128  # edge slots per (window, side)
GROUP_W = 4         # windows fetched per dma_gather pair
SG = 52             # graph one-hot width (50 graphs + 2 junk bins)
KPAD = 32           # padded contraction dim for the post-agg W1 matmul


def _pack_host(x, edge_index, edge_attr, batch):
    """Pure index/layout preprocessing (numpy). Returns per-core input dicts
    plus the static plan and the vid-ordered padded x table."""
    src = np.asarray(edge_index[0], dtype=np.int64)
    dst = np.asarray(edge_index[1], dtype=np.int64)
    ew = np.asarray(edge_attr, dtype=np.float32).reshape(-1)
    loop = np.arange(N_NODES, dtype=np.int64)
    src = np.concatenate([src, loop]).astype(np.int32)
    dst = np.concatenate([dst, loop]).astype(np.int32)
    ew = np.concatenate([ew, np.ones(N_NODES, np.float32)])
    E = src.shape[0]

    deg_cnt = np.bincount(dst, minlength=N_NODES).astype(np.int64)
    node_ptr = np.zeros(N_NODES + 1, np.int64)
    np.cumsum(deg_cnt, out=node_ptr[1:])
    order = np.argsort(dst, kind="stable")

    # core node boundaries balancing edge counts
    cum = node_ptr[1:]
    nb = [0]
    for c in range(1, N_CORES):
        nb.append(int(np.searchsorted(cum, c * E / N_CORES)))
    nb.append(N_NODES)
    nb = np.array(nb, np.int64)
    split_node = int(nb[4])  # src < split_node -> "lo" half of virtual space

    side_lo = src < split_node
    deg_lo = np.bincount(dst[side_lo], minlength=N_NODES).astype(np.int64)
    deg_hi = deg_cnt - deg_lo

    # window packing per core
    core_windows = []
    for c in range(N_CORES):
        wlist = []
        v = int(nb[c])
        end = int(nb[c + 1])
        while v < end:
            ws = v
            lo = hi = cnt = 0
            while (
                v < end
                and cnt < SLOTS
                and lo + deg_lo[v] <= CAP
                and hi + deg_hi[v] <= CAP
            ):
                lo += int(deg_lo[v])
                hi += int(deg_hi[v])
                cnt += 1
                v += 1
            wlist.append((ws, v))
        core_windows.append(wlist)

    NW = max(len(w) for w in core_windows)
    NW = (NW + 7) // 8 * 8  # multiple of GROUP_W; keeps chunking simple
    assert NW <= 128
    NVC = NW * SLOTS
    NV = N_CORES * NVC
    NVH = NV // 2

    # vid map (node -> virtual id), SLOT-MAJOR within a core: s*NW + w
    node_w = np.zeros(N_NODES, np.int32)
    node_s = np.zeros(N_NODES, np.int32)
    node_c = np.zeros(N_NODES, np.int32)
    for c in range(N_CORES):
        for w, (ws, we) in enumerate(core_windows[c]):
            node_c[ws:we] = c
            node_w[ws:we] = w
            node_s[ws:we] = np.arange(we - ws, dtype=np.int32)
    node_vid = node_c * NVC + node_s * NW + node_w

    DEG_K = int(deg_cnt.max())
    DEG_K = (DEG_K + 3) // 4 * 4

    # per-side dst-sorted edge lists + ptrs
    lo_edges = order[side_lo[order]]
    hi_edges = order[~side_lo[order]]
    lo_ptr = np.zeros(N_NODES + 1, np.int64)
    np.cumsum(deg_lo, out=lo_ptr[1:])
    hi_ptr = np.zeros(N_NODES + 1, np.int64)
    np.cumsum(deg_hi, out=hi_ptr[1:])

    NTILES = NW * 2 * T_SIDE
    NG = NW // GROUP_W
    GI = GROUP_W * CAP          # idxs per gather call
    IDXC = GI // 16

    vid_src = node_vid[src]

    # ---- layer-2 packed-pair tiling: 2 nodes per 256B bf16 row.
    # Tiles are segregated by src-vid parity: the first T2S0 tiles of a
    # (window, side) run hold parity-0 edges (matmul reads column half 0),
    # the next T2S1 tiles hold parity-1 edges (column half 1).
    T2S0 = T2S1 = 0
    for c in range(N_CORES):
        for ws, we in core_windows[c]:
            for edges, ptr in ((lo_edges, lo_ptr), (hi_edges, hi_ptr)):
                ids = edges[ptr[ws] : ptr[we]]
                par = vid_src[ids] & 1
                n1 = int(par.sum())
                n0 = ids.shape[0] - n1
                T2S0 = max(T2S0, (n0 + 127) // 128)
                T2S1 = max(T2S1, (n1 + 127) // 128)
    T2S = T2S0 + T2S1
    NTILES2 = NW * 2 * T2S
    GI2 = GROUP_W * T2S * 128
    IDXC2 = GI2 // 16

    # global position of each edge within its dst node's sorted run
    col_within = np.empty(E, np.int64)
    ar = np.arange(E, dtype=np.int64)
    col_within[order] = ar - node_ptr[dst[order]]

    # x in padded vid layout (shared by all cores); cols 5..63 stay zero
    xv = np.zeros((NV, HID), np.float32)
    xv[node_vid, 0:IN_F] = np.asarray(x, np.float32)

    batch_i = np.asarray(batch, np.int64)

    import ml_dtypes

    bfloat16 = ml_dtypes.bfloat16

    per_core = []
    for c in range(N_CORES):
        wlist = core_windows[c]
        ewp = np.zeros((NTILES, 128), np.float32)
        dstp = np.zeros((NTILES, 128), np.float32)
        idx_lo = np.zeros((NW, CAP), np.int16)
        idx_hi = np.zeros((NW, CAP), np.int16)
        ewp2 = np.zeros((NTILES2, 128), np.float32)
        dstp2 = np.zeros((NTILES2, 128), np.float32)
        idx2_lo = np.zeros((NW, T2S * 128), np.int16)
        idx2_hi = np.zeros((NW, T2S * 128), np.int16)
        gid = np.full((SLOTS, NW), 50.0, np.float32)
        ewdeg = np.zeros((NVC, DEG_K), np.float32)

        for w, (ws, we) in enumerate(wlist):
            for s, (edges, ptr, idxbuf, i2buf, voff) in enumerate(
                (
                    (lo_edges, lo_ptr, idx_lo, idx2_lo, 0),
                    (hi_edges, hi_ptr, idx_hi, idx2_hi, NVH),
                )
            ):
                ids = edges[ptr[ws] : ptr[we]]
                n = ids.shape[0]
                t0 = (w * 2 + s) * T_SIDE
                flat_ew = ewp.reshape(-1)
                flat_dst = dstp.reshape(-1)
                base = t0 * 128
                flat_ew[base : base + n] = ew[ids]
                flat_dst[base : base + n] = (dst[ids] - ws).astype(np.float32)
                idxbuf[w, :n] = (vid_src[ids] - voff).astype(np.int16)

                # layer-2 pairing: parity-0 edges fill tiles [0, T2S0),
                # parity-1 edges fill tiles [T2S0, T2S) of this run
                t20 = (w * 2 + s) * T2S
                for gpar, toff in ((0, 0), (1, T2S0)):
                    gids_e = ids[(vid_src[ids] & 1) == gpar]
                    ng = gids_e.shape[0]
                    for ti in range((ng + 127) // 128):
                        ce = gids_e[ti * 128 : ti * 128 + 128]
                        m = ce.shape[0]
                        tt = t20 + toff + ti
                        ewp2[tt, :m] = ew[ce]
                        dstp2[tt, :m] = (dst[ce] - ws).astype(np.float32)
                        i2buf[
                            w, (toff + ti) * 128 : (toff + ti) * 128 + m
                        ] = ((vid_src[ce] - voff) >> 1).astype(np.int16)
            gid[: we - ws, w] = batch_i[ws:we].astype(np.float32)

        # padded per-node edge weights for the degree pass (vid-major rows)
        e_lo = int(node_ptr[nb[c]])
        e_hi = int(node_ptr[nb[c + 1]])
        es = order[e_lo:e_hi]
        rows = node_s[dst[es]] * NW + node_w[dst[es]]
        ewdeg[rows, col_within[es]] = ew[es]
        real = np.zeros(NVC, bool)
        nr = node_s[nb[c] : nb[c + 1]] * NW + node_w[nb[c] : nb[c + 1]]
        real[nr] = True
        ewdeg[~real, 0] = 1.0  # padded slots get deg 1 -> dinv 1 (harmless)

        # wrap gather indices: [NG, 128, *] (16-partition wrap, replicated)
        def wrap(a, gi):
            g = a.reshape(NG, gi // 16, 16).transpose(0, 2, 1)
            return np.ascontiguousarray(np.tile(g, (1, 8, 1)))

        per_core.append(
            dict(
                ewcols=np.ascontiguousarray(ewp.T),
                dstcols=np.ascontiguousarray(dstp.T),
                idxlo=wrap(idx_lo.reshape(-1), GI),
                idxhi=wrap(idx_hi.reshape(-1), GI),
                ewcols2=np.ascontiguousarray(ewp2.T),
                dstcols2=np.ascontiguousarray(dstp2.T),
                idx2lo=wrap(idx2_lo.reshape(-1), GI2),
                idx2hi=wrap(idx2_hi.reshape(-1), GI2),
                gid=np.ascontiguousarray(gid),
                ewdeg=ewdeg,
            )
        )

    plan = dict(
        NW=NW, NVC=NVC, NV=NV, NVH=NVH, DEG_K=DEG_K,
        NTILES=NTILES, NG=NG, GI=GI, IDXC=IDXC,
        T2S=T2S, T2S0=T2S0, NTILES2=NTILES2, GI2=GI2, IDXC2=IDXC2,
    )
    ewdegf = np.concatenate([pc["ewdeg"] for pc in per_core], axis=0)
    return per_core, plan, xv, ewdegf


def _build_program(plan):
    import concourse.bacc as bacc
    import concourse.bass as bass
    import concourse.tile as tile
    from concourse import mybir
    from concourse.tile_rust import add_dep_helper

    f32 = mybir.dt.float32
    bf16 = mybir.dt.bfloat16
    i16 = mybir.dt.int16
    Alu = mybir.AluOpType
    Act = mybir.ActivationFunctionType

    NW = plan["NW"]; NVC = plan["NVC"]; NV = plan["NV"]; NVH = plan["NVH"]
    DEG_K = plan["DEG_K"]; NTILES = plan["NTILES"]; NG = plan["NG"]
    GI = plan["GI"]; IDXC = plan["IDXC"]
    T2S = plan["T2S"]; T2S0 = plan["T2S0"]; NTILES2 = plan["NTILES2"]
    GI2 = plan["GI2"]; IDXC2 = plan["IDXC2"]
    NB = NV // 128          # t1 rows per partition
    NCH = 16                # build chunks
    BC = NB // NCH

    STAGE = int(os.environ.get("KSTAGE", "9"))
    NQ = int(os.environ.get("KNQ", "4"))
    KSP = bool(int(os.environ.get("KSP", "0")))
    nc = bacc.Bacc("TRN2", target_bir_lowering=False, debug=False,
                   num_devices=N_CORES, num_swdge_queues=NQ)

    xvp = nc.declare_dram_parameter("xv", [NV, HID], f32, isOutput=False)
    w1 = nc.declare_dram_parameter("w1", [KPAD, HID], f32, isOutput=False)
    w2 = nc.declare_dram_parameter("w2", [HID, HID], f32, isOutput=False)
    wo = nc.declare_dram_parameter("wo", [HID, OUT_F], f32, isOutput=False)
    b2 = nc.declare_dram_parameter("b2", [SLOTS, HID], f32, isOutput=False)
    bo = nc.declare_dram_parameter("bo", [N_GRAPHS, OUT_F], f32, isOutput=False)
    ewdeg = nc.declare_dram_parameter("ewdeg", [NVC, DEG_K], f32, isOutput=False)
    ewdegf = nc.declare_dram_parameter("ewdegf", [NV, DEG_K], f32, isOutput=False)
    ewcols = nc.declare_dram_parameter("ewcols", [128, NTILES], f32, isOutput=False)
    dstcols = nc.declare_dram_parameter("dstcols", [128, NTILES], f32, isOutput=False)
    idxlo = nc.declare_dram_parameter("idxlo", [NG, 128, IDXC], i16, isOutput=False)
    idxhi = nc.declare_dram_parameter("idxhi", [NG, 128, IDXC], i16, isOutput=False)
    ewcols2 = nc.declare_dram_parameter("ewcols2", [128, NTILES2], f32, isOutput=False)
    dstcols2 = nc.declare_dram_parameter("dstcols2", [128, NTILES2], f32, isOutput=False)
    idx2lo = nc.declare_dram_parameter("idx2lo", [NG, 128, IDXC2], i16, isOutput=False)
    idx2hi = nc.declare_dram_parameter("idx2hi", [NG, 128, IDXC2], i16, isOutput=False)
    gidp = nc.declare_dram_parameter("gid", [SLOTS, NW], f32, isOutput=False)
    out = nc.declare_dram_parameter("out", [N_GRAPHS, OUT_F], f32, isOutput=True)
    chain_in = nc.declare_dram_parameter("chain", [1, 4], f32, isOutput=False)
    chain_out = nc.declare_dram_parameter("chain_out", [1, 4], f32, isOutput=True)
    KDBG = int(os.environ.get("KDBG", "0"))
    if KDBG:
        dbg_dinv = nc.declare_dram_parameter("dbg_dinv", [SLOTS, NW], f32, isOutput=True)
        dbg_t1 = nc.declare_dram_parameter("dbg_t1", [2048, HID], f32, isOutput=True)
        dbg_h1 = nc.declare_dram_parameter("dbg_h1", [2048, HID], bf16, isOutput=True)
        dbg_pool = nc.declare_dram_parameter("dbg_pool", [HID + 1, SG], f32, isOutput=True)

    groups = [list(range(N_CORES))]

    with tile.TileContext(nc) as tc:
        with (
            tc.tile_pool(name="dram", bufs=1, space="DRAM") as dram,
            tc.tile_pool(name="const", bufs=1) as cpool,
            tc.tile_pool(name="persist", bufs=1) as ppool,
        ):
            t1 = dram.tile([NV, HID], f32, tag="t1")
            h1loc = dram.tile([NVC, HID], bf16, tag="h1loc")
            h1glob = dram.tile([NV, HID], bf16, tag="h1glob")
            pool_in_d = dram.tile([HID + 1, SG], f32, tag="poolin")
            pool_out_d = dram.tile([HID + 1, SG], f32, tag="poolout")

            # ---- constants
            iota64 = cpool.tile([128, SLOTS], f32, tag="iota64")
            nc.gpsimd.iota(iota64[:], pattern=[[1, SLOTS]], base=0,
                           channel_multiplier=0,
                           allow_small_or_imprecise_dtypes=True)
            iota52 = cpool.tile([SLOTS, SG], f32, tag="iota52")
            nc.gpsimd.iota(iota52[:], pattern=[[1, SG]], base=0,
                           channel_multiplier=0,
                           allow_small_or_imprecise_dtypes=True)
            w1s = cpool.tile([KPAD, HID], f32, tag="w1s")
            nc.sync.dma_start(w1s[:], w1[:])
            w2s = cpool.tile([HID, HID], f32, tag="w2s")
            nc.sync.dma_start(w2s[:], w2[:])
            wos = cpool.tile([HID, OUT_F], f32, tag="wos")
            nc.sync.dma_start(wos[:], wo[:])
            b2s = cpool.tile([SLOTS, HID], f32, tag="b2s")
            nc.sync.dma_start(b2s[:], b2[:])
            bos = cpool.tile([N_GRAPHS, OUT_F], f32, tag="bos")
            nc.sync.dma_start(bos[:], bo[:])
            gids = cpool.tile([SLOTS, NW], f32, tag="gids")
            nc.sync.dma_start(gids[:], gidp[:])
            ewc = cpool.tile([128, NTILES], f32, tag="ewc")
            nc.sync.dma_start(ewc[:], ewcols[:])
            dstc = cpool.tile([128, NTILES], f32, tag="dstc")
            nc.sync.dma_start(dstc[:], dstcols[:])
            ewc2 = cpool.tile([128, NTILES2], f32, tag="ewc2")
            nc.sync.dma_start(ewc2[:], ewcols2[:])
            dstc2 = cpool.tile([128, NTILES2], f32, tag="dstc2")
            nc.sync.dma_start(dstc2[:], dstcols2[:])
            iota64b = cpool.tile([128, SLOTS], bf16, tag="iota64b")
            nc.vector.tensor_copy(iota64b[:], iota64[:])

            dinvw = ppool.tile([SLOTS, NW], f32, tag="dinvw")

            # rotating pre-transpose staging tiles for the layer-1 epilogue
            NTR = 4
            trs = []
            for j in range(NTR):
                t = ppool.tile([SLOTS, KPAD], f32, tag=f"tr{j}")
                nc.vector.memset(t[:, IN_F:KPAD], 0.0)
                nc.vector.memset(t[:, KPAD - 1 : KPAD], 1.0)  # bias row hook
                trs.append(t)

            KCC = int(os.environ.get("KCC", "1"))
            KNG = int(os.environ.get("KNG", "9999"))
            KGATHER = int(os.environ.get("KGATHER", "1"))
            KAMP = int(os.environ.get("KAMP", "1"))

            # ---- message-passing layer (one pass over the edge tiles)
            def layer(l, rep, src_table, fence, h2stage):
                if l == 1:
                    lo_view = src_table[0:NVH, :]
                    hi_view = src_table[NVH:NV, :]
                    TS, GIL, IDXCL = T_SIDE, GI, IDXC
                    ixlo, ixhi = idxlo, idxhi
                    melem, mdt = HID, f32
                else:
                    packed = src_table[:].rearrange("(a b) h -> a (b h)", b=2)
                    lo_view = packed[0 : NVH // 2, :]
                    hi_view = packed[NVH // 2 : NV // 2, :]
                    TS, GIL, IDXCL = T2S, GI2, IDXC2
                    ixlo, ixhi = idx2lo, idx2hi
                    melem, mdt = 2 * HID, bf16
                KBF16 = int(os.environ.get("KBF16", "1"))
                sdt = bf16 if (l == 2 or KBF16) else f32
                iot = iota64b if sdt == bf16 else iota64
                KMB = int(os.environ.get("KMB", "2"))
                KSB = int(os.environ.get("KSB", "6"))
                KIB = int(os.environ.get("KIB", "4"))
                KWB = int(os.environ.get("KWB", "4"))
                KEB = int(os.environ.get("KEB", "3"))
                with (
                    tc.tile_pool(name=f"idx{l}_{rep}", bufs=KIB) as ipool,
                    tc.tile_pool(name=f"mbuf{l}_{rep}", bufs=KMB) as mpool,
                    tc.tile_pool(name=f"sbld{l}_{rep}", bufs=KSB) as spool,
                    tc.tile_pool(name=f"wpsum{l}_{rep}", bufs=KWB, space="PSUM") as wpool,
                    tc.tile_pool(name=f"epi{l}_{rep}", bufs=KEB) as epool,
                    tc.tile_pool(name=f"p2_{l}_{rep}", bufs=2, space="PSUM") as p2pool,
                    tc.tile_pool(name=f"gpool{l}_{rep}", bufs=1, space="PSUM") as gpool,
                ):
                    if l == 2:
                        pool_ps = gpool.tile([HID + 1, SG], f32, tag="poolps")
                    for g in range(min(NG, KNG)):
                        ilo = ipool.tile([128, IDXCL], i16, tag="ilo")
                        nc.sync.dma_start(ilo[:], ixlo[g])
                        ihi = ipool.tile([128, IDXCL], i16, tag="ihi")
                        nc.sync.dma_start(ihi[:], ixhi[g])
                        mlo = mpool.tile([128, GROUP_W * TS, melem], mdt,
                                         tag="mlo")
                        mhi = mpool.tile([128, GROUP_W * TS, melem], mdt,
                                         tag="mhi")
                        KSPLIT = int(os.environ.get("KSPLIT", "0"))
                        gathers = []
                        if KSPLIT:
                            hb = GROUP_W * TS // 2
                            hi_idx = GIL // 2
                            hc = IDXCL // 2
                            for j, (mt, view, it) in enumerate(
                                (
                                    (mlo[:, 0:hb, :], lo_view, ilo[:, 0:hc]),
                                    (mlo[:, hb:, :], lo_view, ilo[:, hc:]),
                                    (mhi[:, 0:hb, :], hi_view, ihi[:, 0:hc]),
                                    (mhi[:, hb:, :], hi_view, ihi[:, hc:]),
                                )
                            ):
                                gathers.append(
                                    nc.gpsimd.dma_gather(
                                        mt, view, it, hi_idx, hi_idx, melem,
                                        single_packet=KSP,
                                        queue_num=(4 * g + j) % NQ,
                                    )
                                )
                        else:
                            gathers.append(
                                nc.gpsimd.dma_gather(
                                    mlo[:], lo_view, ilo[:], GIL, GIL, melem,
                                    single_packet=KSP,
                                    queue_num=(2 * g) % NQ,
                                )
                            )
                            gathers.append(
                                nc.gpsimd.dma_gather(
                                    mhi[:], hi_view, ihi[:], GIL, GIL, melem,
                                    single_packet=KSP,
                                    queue_num=(2 * g + 1) % NQ,
                                )
                            )
                        if fence is not None:
                            for gg in gathers:
                                add_dep_helper(gg.ins, fence.ins,
                                               reason="gather src table ready")
                        if l == 1 and KBF16:
                            mlob = mpool.tile(
                                [128, GROUP_W * TS, melem], bf16, tag="mlob"
                            )
                            nc.scalar.activation(mlob[:], mlo[:], Act.Copy)
                            mhib = mpool.tile(
                                [128, GROUP_W * TS, melem], bf16, tag="mhib"
                            )
                            nc.scalar.activation(mhib[:], mhi[:], Act.Copy)
                            mlo, mhi = mlob, mhib
                        for wl in range(GROUP_W):
                            w = g * GROUP_W + wl
                            dv = dinvw[:, w : w + 1]
                            if l == 1:
                                ps = wpool.tile([SLOTS, 8], f32, tag="wps")
                            else:
                                ps = wpool.tile([SLOTS, HID], f32, tag="wps")
                            k = 0
                            nmm = 2 * TS
                            for s, mb in ((0, mlo), (1, mhi)):
                                for ti in range(TS):
                                    t = (w * 2 + s) * TS + ti
                                    blk = wl * TS + ti
                                    S = spool.tile([128, SLOTS], sdt, tag="S")
                                    if l == 1:
                                        nc.vector.tensor_scalar(
                                            out=S[:], in0=iot[:],
                                            scalar1=dstc[:, t : t + 1],
                                            scalar2=ewc[:, t : t + 1],
                                            op0=Alu.is_equal, op1=Alu.mult,
                                        )
                                        nc.tensor.matmul(
                                            out=ps[:], lhsT=S[:],
                                            rhs=mb[:, blk, 0:8],
                                            start=(k == 0),
                                            stop=(k == nmm - 1),
                                        )
                                    else:
                                        nc.vector.tensor_scalar(
                                            out=S[:], in0=iot[:],
                                            scalar1=dstc2[:, t : t + 1],
                                            scalar2=ewc2[:, t : t + 1],
                                            op0=Alu.is_equal, op1=Alu.mult,
                                        )
                                        c0 = 0 if ti < T2S0 else HID
                                        nc.tensor.matmul(
                                            out=ps[:],
                                            lhsT=mb[:, blk, c0 : c0 + HID],
                                            rhs=S[:],
                                            start=(k == 0),
                                            stop=(k == nmm - 1),
                                        )
                                    k += 1
                            if l == 1:
                                # agg[s,0:5]*dv -> rotate into tr, transpose,
                                # h1 = relu((tr^T @ W1p)) * dv  (bias row in W1p)
                                tr = trs[w % NTR]
                                nc.vector.tensor_scalar(
                                    out=tr[:, 0:8], in0=ps[:], scalar1=dv,
                                    scalar2=None, op0=Alu.mult,
                                )
                                aggT = epool.tile([KPAD, SLOTS], f32, tag="aggT1")
                                nc.vector.transpose(
                                    aggT[:, 0:32], tr[0:32, :]
                                )
                                nc.vector.transpose(
                                    aggT[:, 32:64], tr[32:64, :]
                                )
                                h1w = p2pool.tile([SLOTS, HID], f32, tag="h1w")
                                nc.tensor.matmul(
                                    out=h1w[:], lhsT=aggT[:], rhs=w1s[:],
                                    start=True, stop=True,
                                )
                                nc.vector.tensor_scalar(
                                    out=h2stage[:, w, :], in0=h1w[:],
                                    scalar1=0.0, scalar2=dv,
                                    op0=Alu.max, op1=Alu.mult,
                                )
                            else:
                                aggT = epool.tile([HID, SLOTS], f32, tag="aggT2")
                                nc.vector.tensor_copy(aggT[:], ps[:])
                                ps2 = p2pool.tile([SLOTS, HID], f32, tag="ps2")
                                nc.tensor.matmul(
                                    out=ps2[:], lhsT=aggT[:], rhs=w2s[:],
                                    start=True, stop=True,
                                )
                                u = epool.tile([SLOTS, HID + 1], f32, tag="u2")
                                nc.vector.memset(u[:, HID : HID + 1], 1.0)
                                nc.vector.tensor_scalar(
                                    out=u[:, 0:HID], in0=ps2[:], scalar1=dv,
                                    scalar2=None, op0=Alu.mult,
                                )
                                nc.vector.tensor_tensor(
                                    out=u[:, 0:HID], in0=u[:, 0:HID],
                                    in1=b2s[:], op=Alu.add,
                                )
                                nc.vector.tensor_scalar(
                                    out=u[:, 0:HID], in0=u[:, 0:HID],
                                    scalar1=0.0, scalar2=None, op0=Alu.max,
                                )
                                Sg = epool.tile([SLOTS, SG], f32, tag="Sg")
                                nc.vector.tensor_scalar(
                                    out=Sg[:], in0=iota52[:],
                                    scalar1=gids[:, w : w + 1],
                                    scalar2=None, op0=Alu.is_equal,
                                )
                                nc.tensor.matmul(
                                    out=pool_ps[:], lhsT=u[:], rhs=Sg[:],
                                    start=(w == 0),
                                    stop=(w == min(NG, KNG) * GROUP_W - 1),
                                )
                    if l == 2 and KNG >= NG:
                        pst = epool.tile([HID + 1, SG], f32, tag="pst")
                        nc.vector.tensor_copy(pst[:], pool_ps[:])
                        nc.sync.dma_start(pool_in_d[:], pst[:])

            # ---- one full pipeline iteration (repeated KAMP x for timing)
            for rep in range(KAMP):
                # degree pass -> local dinv (vid-major ewdeg: contiguous load)
                with tc.tile_pool(name=f"deg{rep}", bufs=1) as dpool:
                    degt = dpool.tile([SLOTS, NW, DEG_K], f32, tag="degt")
                    nc.sync.dma_start(
                        degt[:], ewdeg[:].rearrange("(s w) k -> s w k", s=SLOTS)
                    )
                    deg = dpool.tile([SLOTS, NW], f32, tag="deg")
                    nc.vector.tensor_reduce(
                        out=deg[:], in_=degt[:], axis=mybir.AxisListType.X,
                        op=Alu.add,
                    )
                    rec = dpool.tile([SLOTS, NW], f32, tag="rec")
                    nc.vector.reciprocal(rec[:], deg[:])
                    nc.scalar.activation(dinvw[:], rec[:], Act.Sqrt)

                # build T1 = dinv * xv; dinv computed from the replicated
                # full ewdeg (no collective; all contiguous traffic)
                t1_writes = []
                if STAGE >= 1:
                    with (
                        tc.tile_pool(name=f"tb{rep}", bufs=2) as tbpool,
                    ):
                        xvv = xvp[:].rearrange("(p b) h -> p b h", p=128)
                        edv = ewdegf[:].rearrange("(p b) k -> p b k", p=128)
                        t1v = t1[:].rearrange("(p b) h -> p b h", p=128)
                        for ch in range(NCH):
                            b0 = ch * BC
                            xs = tbpool.tile([128, BC, HID], f32, tag="xs")
                            nc.sync.dma_start(xs[:], xvv[:, b0 : b0 + BC, :])
                            edc = tbpool.tile([128, BC, DEG_K], f32, tag="edc")
                            nc.sync.dma_start(edc[:], edv[:, b0 : b0 + BC, :])
                            dgc = tbpool.tile([128, BC], f32, tag="dgc")
                            nc.vector.tensor_reduce(
                                out=dgc[:], in_=edc[:],
                                axis=mybir.AxisListType.X, op=Alu.add,
                            )
                            rcc = tbpool.tile([128, BC], f32, tag="rcc")
                            nc.vector.reciprocal(rcc[:], dgc[:])
                            dvt = tbpool.tile([128, BC], f32, tag="dvt")
                            nc.scalar.activation(dvt[:], rcc[:], Act.Sqrt)
                            ts = tbpool.tile([128, BC, HID], f32, tag="ts")
                            for b in range(BC):
                                nc.vector.tensor_scalar(
                                    out=ts[:, b, :], in0=xs[:, b, :],
                                    scalar1=dvt[:, b : b + 1],
                                    scalar2=None, op0=Alu.mult,
                                )
                            t1_writes.append(
                                nc.sync.dma_start(
                                    t1v[:, b0 : b0 + BC, :], ts[:]
                                )
                            )

                if STAGE >= 2:
                    fence1 = nc.gpsimd.engine_nop()
                    for wi in t1_writes:
                        add_dep_helper(fence1.ins, wi.ins,
                                       reason="t1 table ready before gathers")
                    with tc.tile_pool(name=f"h2p{rep}", bufs=1) as h2pool:
                        h2stage = h2pool.tile([SLOTS, NW, HID], bf16,
                                              tag="h2stage")
                        layer(1, rep, t1, fence1, h2stage)
                        h1_write = nc.sync.dma_start(
                            h1loc[:].rearrange("(s w) h -> s w h", s=SLOTS),
                            h2stage[:],
                        )

                if STAGE >= 3:
                    cc_h1 = nc.gpsimd.collective_compute(
                        "AllGather", Alu.bypass, replica_groups=groups,
                        ins=[h1loc[:].rearrange("a b -> (a b)")],
                        outs=[h1glob[:].rearrange("a b -> (a b)")],
                    )
                    add_dep_helper(cc_h1.ins, h1_write.ins,
                                   reason="h1 staged before allgather")

                if STAGE >= 4:
                    layer(2, rep, h1glob, cc_h1, None)

            if STAGE < 4:
                # keep the tail runnable: zero the pooled partials
                with tc.tile_pool(name="dummy", bufs=1) as dpool2:
                    osb0 = dpool2.tile([HID + 1, SG], f32, tag="osb0")
                    nc.vector.memset(osb0[:], 0.0)
                    nc.sync.dma_start(pool_in_d[:], osb0[:])

            if KDBG:
                nc.sync.dma_start(dbg_dinv[:], dinvw[:])
                nc.sync.dma_start(dbg_t1[:], t1[0:2048, :])
                nc.sync.dma_start(dbg_h1[:], h1loc[0:2048, :])
                nc.sync.dma_start(dbg_pool[:], pool_in_d[:])

            # ---- pooled partial sums -> all-reduce -> final linear
            if KCC:
                nc.gpsimd.collective_compute(
                    "AllReduce", Alu.add, replica_groups=groups,
                    ins=[pool_in_d[:]], outs=[pool_out_d[:]],
                )
            else:
                nc.sync.dma_start(pool_out_d[:], pool_in_d[:])
            with (
                tc.tile_pool(name="fin", bufs=1) as fpool,
                tc.tile_pool(name="finps", bufs=1, space="PSUM") as fpsum,
            ):
                pr = fpool.tile([HID + 1, SG], f32, tag="pr")
                nc.sync.dma_start(pr[:], pool_out_d[:])
                cm = fpool.tile([1, SG], f32, tag="cm")
                nc.vector.tensor_scalar(
                    out=cm[:], in0=pr[HID : HID + 1, :], scalar1=1.0,
                    scalar2=None, op0=Alu.max,
                )
                rcp = fpool.tile([1, SG], f32, tag="rcp")
                nc.vector.reciprocal(rcp[:], cm[:])
                rcpb = fpool.tile([HID, SG], f32, tag="rcpb")
                nc.gpsimd.partition_broadcast(rcpb[:], rcp[:])
                pooledT = fpool.tile([HID, N_GRAPHS], f32, tag="pooledT")
                nc.vector.tensor_tensor(
                    out=pooledT[:], in0=pr[0:HID, 0:N_GRAPHS],
                    in1=rcpb[0:HID, 0:N_GRAPHS],
                    op=Alu.mult,
                )
                pso = fpsum.tile([N_GRAPHS, OUT_F], f32, tag="pso")
                nc.tensor.matmul(
                    out=pso[:], lhsT=pooledT[:], rhs=wos[:],
                    start=True, stop=True,
                )
                osb = fpool.tile([N_GRAPHS, OUT_F], f32, tag="osb")
                nc.vector.tensor_tensor(
                    out=osb[:], in0=pso[:],
                    in1=bos[:],
                    op=Alu.add,
                )
                nc.sync.dma_start(out[:], osb[:])
                chs = fpool.tile([1, 4], f32, tag="chs")
                nc.sync.dma_start(chs[:], chain_in[:])
                nc.vector.tensor_scalar_add(chs[:], chs[:], 1.0)
                nc.sync.dma_start(chain_out[:], chs[:])

    nc.compile()
    return nc


def _common_inputs(xv, ewdegf, W1, b1, W2, b2, Wo, bo):
    w1p = np.zeros((KPAD, HID), np.float32)
    w1p[0:IN_F] = np.asarray(W1, np.float32)
    w1p[KPAD - 1] = np.asarray(b1, np.float32)  # bias row (tr col KPAD-1 == 1)
    return dict(
        chain=np.zeros((1, 4), np.float32),
        xv=xv,
        ewdegf=ewdegf,
        w1=w1p,
        w2=np.asarray(W2, np.float32),
        wo=np.asarray(Wo, np.float32),
        b2=np.tile(np.asarray(b2, np.float32).reshape(1, -1), (SLOTS, 1)),
        bo=np.tile(np.asarray(bo, np.float32).reshape(1, -1), (N_GRAPHS, 1)),
    )


def _make_runner(nc, repeat=1):
    """Cached-jit SPMD runner modeled on bass2jax.run_bass_via_pjrt, for
    benchmarking: returns (fn, prep, unpack)."""
    import jax
    import numpy as np
    from jax.experimental.shard_map import shard_map
    from jax.sharding import Mesh, NamedSharding, PartitionSpec

    from concourse import bass2jax, mybir
    from concourse.bass2jax import (
        _bass_exec_p, install_neuronx_cc_hook, partition_id_tensor,
    )

    install_neuronx_cc_hook()
    pname = nc.partition_id_tensor.name if nc.partition_id_tensor else None
    in_names, out_names, out_avals, zero_outs = [], [], [], []
    for alloc in nc.m.functions[0].allocations:
        if not isinstance(alloc, mybir.MemoryLocationSet):
            continue
        name = alloc.memorylocations[0].name
        if alloc.kind == "ExternalInput":
            if name == pname:
                continue
            in_names.append(name)
        elif alloc.kind == "ExternalOutput":
            shape = tuple(alloc.tensor_shape)
            dtype = mybir.dt.np(alloc.dtype)
            out_names.append(name)
            out_avals.append(jax.core.ShapedArray(shape, dtype))
            zero_outs.append(np.zeros(shape, dtype))
    n_params = len(in_names)
    all_names = in_names + out_names
    if pname is not None:
        all_names = all_names + [pname]

    def _body(*args):
        operands = list(args)
        if pname is not None:
            operands.append(partition_id_tensor())
        outs = _bass_exec_p.bind(
            *operands,
            out_avals=tuple(out_avals),
            in_names=tuple(all_names),
            out_names=tuple(out_names),
            lowering_input_output_aliases=(),
            sim_require_finite=True,
            sim_require_nnan=True,
            nc=nc,
        )
        return tuple(outs)

    devices = jax.devices()[:N_CORES]
    mesh = Mesh(np.asarray(devices), ("core",))
    spec = PartitionSpec("core")
    n_all = n_params + len(out_names)
    fn = jax.jit(
        shard_map(
            _body, mesh=mesh, in_specs=(spec,) * n_all,
            out_specs=(spec,) * len(out_names), check_rep=False,
        ),
        keep_unused=True,
    )

    def prep(in_maps):
        sharding = NamedSharding(mesh, spec)
        args = []
        for i, name in enumerate(in_names):
            cat = np.concatenate([np.asarray(m[name]) for m in in_maps], axis=0)
            args.append(jax.device_put(cat, sharding))
        for z in zero_outs:
            cat = np.zeros((N_CORES * z.shape[0], *z.shape[1:]), z.dtype)
            args.append(jax.device_put(cat, sharding))
        return args

    def unpack(outs):
        return {
            name: np.asarray(outs[i]).reshape(N_CORES, *out_avals[i].shape)[0]
            for i, name in enumerate(out_names)
        }

    return fn, prep, unpack


def kernel(x, edge_index, edge_attr, batch, W1, b1, W2, b2, Wo, bo, **_):
    per_core, plan, xv, ewdegf = _pack_host(x, edge_index, edge_attr, batch)
    nc = _build_program(plan)

    common = _common_inputs(xv, ewdegf, W1, b1, W2, b2, Wo, bo)
    in_maps = []
    for c in range(N_CORES):
        m = dict(common)
        m.update(per_core[c])
        in_maps.append(m)

    from concourse.bass_utils import run_bass_kernel_spmd

    res = run_bass_kernel_spmd(nc, in_maps, list(range(N_CORES)))
    out = res.results[0]["out"]
    kernel.last_exec_time_ns = res.exec_time_ns
    kernel.last_results = res.results
    return np.asarray(out, np.float32)


kernel.last_exec_time_ns = None
